# revision 10
# baseline (speedup 1.0000x reference)
"""Trainium2 Bass kernel for nn_BaseEvaluator_13391708029993.

Per batch: for each of N=16 offset candidates, bilinearly sample features at
(x+ox, y+oy) (clipped; mirrors the reference's XLA-traced normalize roundtrip
including its reciprocal-multiply + fma edge behavior), compute grouped-channel
means of -|f - warped| for channel rolls {0,8,16}, max over the 12 groups ->
strength; temperature-1000 softmax over the 16 candidates weights the offsets;
output clip(weighted + coord) - coord.

Sharding: 8 cores = (4 batches) x (2 row-halves); no cross-core communication.

Device pipeline per core:
  Phase A: PE-transpose features [32, HW] into a row-pair-interleaved gather
           layout fpj[q=(y*W+x)] = [F[y,x,:], F[min(y+1,H-1),x,:]] (64 f32).
  Phase B: per 8-row group: PE-transpose offset slabs to [pixel, n] layout,
           compute indices/weights on DVE; per (row-pair, x-half) block one
           indirect-DMA gather (512B descriptors = all 4 bilinear corners x 32
           channels), then DVE bilinear / |diff| group-sums / softmax.
"""
import numpy as np

import concourse.bacc as bacc
import concourse.bass as bass
import concourse.mybir as mybir
import concourse.tile as tile

F32 = mybir.dt.float32
I32 = mybir.dt.int32
ALU = mybir.AluOpType
ACTF = mybir.ActivationFunctionType
AXL = mybir.AxisListType

H = W = 256
C = 32
N = 16
HW = H * W
NCORES = 8

C127 = np.float32(1.0) / np.float32(127.5)
K127 = np.float32(1.0 - np.float64(127.5) * np.float64(C127))


def _ap(t, off, dims):
    return bass.AP(t, off, [list(d) for d in dims])


def _fr(ap, dims, extra_off=0):
    """Replace the free dims of an SBUF/PSUM AP (keeps partition dim)."""
    return bass.AP(ap.tensor, ap.offset + extra_off,
                   [list(ap.ap[0])] + [list(d) for d in dims])


def build_module(dbg=False):
    nc = bacc.Bacc("TRN2", target_bir_lowering=False, debug=False,
                   enable_asserts=False, num_devices=1)

    feat = nc.dram_tensor("feat", [C, HW], F32, kind="ExternalInput")
    offx = nc.dram_tensor("offx", [N, 128, W], F32, kind="ExternalInput")
    offy = nc.dram_tensor("offy", [N, 128, W], F32, kind="ExternalInput")
    rbase_in = nc.dram_tensor("rbase", [128, 1], F32, kind="ExternalInput")
    outx = nc.dram_tensor("outx", [128, W], F32, kind="ExternalOutput")
    outy = nc.dram_tensor("outy", [128, W], F32, kind="ExternalOutput")
    fp2 = nc.dram_tensor("fp2", [HW, 4 * C], F32,
                         kind="ExternalOutput" if dbg else "Internal")
    if dbg:
        d_idx = nc.dram_tensor("d_idx", [128, 256], I32, kind="ExternalOutput")
        d_g2 = nc.dram_tensor("d_g2", [128, 4096], F32, kind="ExternalOutput")
        d_smin = nc.dram_tensor("d_smin", [128, 32], F32, kind="ExternalOutput")
        d_wa = nc.dram_tensor("d_wa", [128, 256], F32, kind="ExternalOutput")

    ident_t = nc.inline_tensor(np.eye(128, dtype=np.float32), name="ident128")
    xoff_np = np.broadcast_to(
        (np.arange(128, dtype=np.float32)[:, None, None, None]
         + 128.0 * np.arange(2, dtype=np.float32)[None, None, :, None]),
        (128, 8, 2, N)).reshape(128, 256).copy()
    xoff_t = nc.inline_tensor(xoff_np, name="xoffc")
    yrel_np = np.broadcast_to(np.arange(8, dtype=np.float32)[None, :, None, None],
                              (128, 8, 2, N)).reshape(128, 256).copy()
    yrel_t = nc.inline_tensor(yrel_np, name="yrelc")
    xcol_np = np.arange(128, dtype=np.float32)[:, None]
    xc0_t = nc.inline_tensor(xcol_np.copy(), name="xcol0")
    xc1_t = nc.inline_tensor(xcol_np + 128.0, name="xcol1")
    pbase_np = (np.arange(8, dtype=np.float32)[None, :, None] * W
                + np.arange(2, dtype=np.float32)[None, None, :] * 128
                + np.arange(128, dtype=np.float32)[:, None, None]).reshape(128, 16).copy()
    pbase_t = nc.inline_tensor(pbase_np, name="pbasec")

    with tile.TileContext(nc) as tc:
        with (
            tc.tile_pool(name="consts", bufs=1) as cpool,
            tc.tile_pool(name="psA", bufs=2, space="PSUM") as psA,
            tc.tile_pool(name="psB", bufs=2, space="PSUM") as psB,
            tc.tile_pool(name="ixs", bufs=1) as ixspool,
            tc.tile_pool(name="ixk", bufs=2) as ixkpool,
            tc.tile_pool(name="gat", bufs=2) as gatpool,
            tc.tile_pool(name="cmp", bufs=2) as cmppool,
            tc.tile_pool(name="sm", bufs=2) as smpool,
            tc.tile_pool(name="outp", bufs=1) as outpool,
        ):
            idn = cpool.tile([128, 128], F32, tag="ident")
            nc.sync.dma_start(out=idn[:], in_=ident_t.ap())
            xoff = cpool.tile([128, 256], F32, tag="xoff")
            nc.sync.dma_start(out=xoff[:], in_=xoff_t.ap())
            yrel = cpool.tile([128, 256], F32, tag="yrel")
            nc.sync.dma_start(out=yrel[:], in_=yrel_t.ap())
            xc0 = cpool.tile([128, 1], F32, tag="xc0")
            nc.sync.dma_start(out=xc0[:], in_=xc0_t.ap())
            xc1 = cpool.tile([128, 1], F32, tag="xc1")
            nc.sync.dma_start(out=xc1[:], in_=xc1_t.ap())
            pbase = cpool.tile([128, 16], F32, tag="pbase")
            nc.sync.dma_start(out=pbase[:], in_=pbase_t.ap())
            rbase = cpool.tile([128, 1], F32, tag="rbase")
            nc.sync.dma_start(out=rbase[:], in_=rbase_in.ap())
            rb256 = cpool.tile([128, 1], F32, tag="rb256")
            nc.vector.tensor_scalar(out=rb256[:], in0=rbase[:], scalar1=float(W),
                                    scalar2=None, op0=ALU.mult)

            zpad = cpool.tile([128, 2 * C], F32, tag="zpad")
            nc.vector.memset(zpad[:], 0.0)
            # last-row blocks (255,x): (i1,j1) slot at offset 96 never written
            nc.sync.dma_start(
                out=_ap(fp2, (HW - 256) * 128 + 96,
                        [[128, 128], [16384, 2], [1, C]]),
                in_=_ap(zpad[:].tensor, zpad[:].offset, [list(zpad[:].ap[0]), [32, 2], [1, C]]))
            # block (254,255) offset 96 and block (255,255) offset 64
            nc.sync.dma_start(out=_ap(fp2, 65279 * 128 + 96, [[1, 1], [1, C]]),
                              in_=zpad[:1, :C])
            nc.sync.dma_start(out=_ap(fp2, 65535 * 128 + 64, [[1, 1], [1, C]]),
                              in_=zpad[:1, :C])

            # ---------------- Phase A: build fpj ----------------
            with (
                tc.tile_pool(name="ldA", bufs=2) as ldApool,
                tc.tile_pool(name="tpA", bufs=3) as tpApool,
            ):
                for t in range(16):
                    ftile = ldApool.tile([C, 4096], F32, tag="ftile")
                    nc.sync.dma_start(out=ftile[:],
                                      in_=feat.ap()[:, t * 4096:(t + 1) * 4096])
                    for half in range(2):
                        pt = psA.tile([128, 16, C], F32, tag="pt")
                        for u in range(16):
                            uu = half * 16 + u
                            nc.tensor.transpose(
                                out=pt[:, u, :],
                                in_=ftile[:, uu * 128:(uu + 1) * 128],
                                identity=idn[:C, :C])
                        tt = tpApool.tile([128, 16, C], F32, tag="tt")
                        nc.vector.tensor_copy(out=tt[:], in_=pt[:])
                        base = t * 4096 + half * 2048
                        AP3 = lambda off, nu: _ap(fp2, off,
                                                  [[128, 128], [16384, nu], [1, C]])
                        # (i0,j0): block q, offset 0
                        nc.sync.dma_start(out=AP3(base * 128, 16), in_=tt[:])
                        # (i0,j1): block q-256, offset 32
                        if base == 0:
                            nc.sync.dma_start(out=AP3(32, 14), in_=tt[:, 2:16, :])
                        else:
                            nc.sync.dma_start(out=AP3((base - 256) * 128 + 32, 16),
                                              in_=tt[:])
                        # (i1,j0): block q-1, offset 64
                        if base == 0:
                            nc.sync.dma_start(
                                out=_ap(fp2, 64, [[128, 127], [1, C]]),
                                in_=tt[1:128, 0, :])
                            nc.sync.dma_start(out=AP3(127 * 128 + 64, 15),
                                              in_=tt[:, 1:16, :])
                        else:
                            nc.sync.dma_start(out=AP3((base - 1) * 128 + 64, 16),
                                              in_=tt[:])
                        # (i1,j1): block q-257, offset 96
                        if base == 0:
                            nc.sync.dma_start(
                                out=_ap(fp2, 96, [[128, 127], [1, C]]),
                                in_=tt[1:128, 2, :])
                            nc.sync.dma_start(out=AP3(127 * 128 + 96, 13),
                                              in_=tt[:, 3:16, :])
                        else:
                            nc.sync.dma_start(out=AP3((base - 257) * 128 + 96, 16),
                                              in_=tt[:])
                        # clamp fills for last row (j=1 slots read row 255 itself)
                        if t == 15 and half == 1:
                            nc.sync.dma_start(out=AP3((HW - 256) * 128 + 32, 2),
                                              in_=tt[:, 14:16, :])
                            nc.sync.dma_start(out=AP3((HW - 257) * 128 + 96, 2),
                                              in_=tt[:, 14:16, :])

            # ---------------- Phase B ----------------
            OUTT = {}
            for hh in range(2):
                OUTT[('x', hh)] = outpool.tile([128, 128], F32, tag=f"ox{hh}",
                                               name=f"otx{hh}")
                OUTT[('y', hh)] = outpool.tile([128, 128], F32, tag=f"oy{hh}",
                                               name=f"oty{hh}")

            def ts(out, in0, s1, s2, op0, op1=None):
                kw = {}
                if op1 is not None:
                    kw['op1'] = op1
                nc.vector.tensor_scalar(out=out, in0=in0, scalar1=s1,
                                        scalar2=s2, op0=op0, **kw)

            def tt_(out, in0, in1, op):
                nc.vector.tensor_tensor(out=out, in0=in0, in1=in1, op=op)

            KEPT = {"YF", "IDX2", "FID2", "WA", "WB", "WC", "WD", "OX", "OY"}

            def newt(tag, shape=(128, 256), dt=F32):
                pool = ixkpool if tag in KEPT else ixspool
                return pool.tile(list(shape), dt, tag=tag, name=tag)

            with tc.tile_pool(name="ldB", bufs=2) as ldBpool:
                for g in range(16):
                    oxs = ldBpool.tile([N, 8, W], F32, tag="oxs")
                    nc.sync.dma_start(out=oxs[:],
                                      in_=offx.ap()[:, g * 8:(g + 1) * 8, :])
                    oys = ldBpool.tile([N, 8, W], F32, tag="oys")
                    nc.sync.dma_start(out=oys[:],
                                      in_=offy.ap()[:, g * 8:(g + 1) * 8, :])

                    pox = psB.tile([128, 8, 2, N], F32, tag="pox")
                    poy = psB.tile([128, 8, 2, N], F32, tag="poy")
                    for yy in range(8):
                        for hh in range(2):
                            nc.tensor.transpose(
                                out=pox[:, yy, hh, :],
                                in_=oxs[:, yy, hh * 128:(hh + 1) * 128],
                                identity=idn[:N, :N])
                            nc.tensor.transpose(
                                out=poy[:, yy, hh, :],
                                in_=oys[:, yy, hh * 128:(hh + 1) * 128],
                                identity=idn[:N, :N])
                    OX = newt("OX")
                    nc.vector.tensor_copy(out=OX[:], in_=_fr(pox[:], [[1, 256]]))
                    OY = newt("OY")
                    nc.vector.tensor_copy(out=OY[:], in_=_fr(poy[:], [[1, 256]]))

                    # y-full coordinate (exact integer adds)
                    YF = newt("YF")
                    ts(YF[:], yrel[:], rbase[:], float(g * 8), ALU.add, ALU.add)

                    # x side
                    RX = newt("RX")
                    tt_(RX[:], OX[:], xoff[:], ALU.add)
                    ts(RX[:], RX[:], float(W - 1), 0.0, ALU.min, ALU.max)
                    IX = newt("IX")
                    ts(IX[:], RX[:], float(C127), float(K127), ALU.mult, ALU.add)
                    ts(IX[:], IX[:], 127.5, None, ALU.mult)
                    XRI = newt("XRI", dt=I32)
                    nc.vector.tensor_copy(out=XRI[:], in_=IX[:])
                    XR = newt("XR")
                    nc.vector.tensor_copy(out=XR[:], in_=XRI[:])
                    FIXX = newt("FIXX")
                    tt_(FIXX[:], IX[:], XR[:], ALU.is_lt)
                    X0 = newt("X0")
                    tt_(X0[:], XR[:], FIXX[:], ALU.subtract)
                    WX = newt("WX")
                    tt_(WX[:], IX[:], X0[:], ALU.subtract)
                    ts(X0[:], X0[:], 0.0, float(W - 1), ALU.max, ALU.min)

                    # y side
                    RY = newt("RY")
                    tt_(RY[:], OY[:], YF[:], ALU.add)
                    ts(RY[:], RY[:], float(H - 1), 0.0, ALU.min, ALU.max)
                    IY = newt("IY")
                    ts(IY[:], RY[:], float(C127), float(K127), ALU.mult, ALU.add)
                    ts(IY[:], IY[:], 127.5, None, ALU.mult)
                    YRI = newt("YRI", dt=I32)
                    nc.vector.tensor_copy(out=YRI[:], in_=IY[:])
                    YR = newt("YR")
                    nc.vector.tensor_copy(out=YR[:], in_=YRI[:])
                    FIXY = newt("FIXY")
                    tt_(FIXY[:], IY[:], YR[:], ALU.is_lt)
                    Y0 = newt("Y0")
                    tt_(Y0[:], YR[:], FIXY[:], ALU.subtract)
                    WY = newt("WY")
                    tt_(WY[:], IY[:], Y0[:], ALU.subtract)
                    ts(Y0[:], Y0[:], 0.0, float(H - 1), ALU.max, ALU.min)

                    # gather indices in block-contiguous layout [128,(hh,jj,rr,n)]
                    IDXF = newt("IDXF")
                    nc.vector.scalar_tensor_tensor(out=IDXF[:], in0=Y0[:],
                                                   scalar=float(W), in1=X0[:],
                                                   op0=ALU.mult, op1=ALU.add)
                    IDX2 = newt("IDX2", dt=I32)
                    nc.vector.tensor_copy(
                        out=_fr(IDX2[:], [[32, 4], [16, 2], [128, 2], [1, 16]]),
                        in_=IDXF[:])
                    FIDF = newt("FIDF", shape=(128, 16))
                    ts(FIDF[:], pbase[:], rb256[:], float(g * 8 * W),
                       ALU.add, ALU.add)
                    FID2 = newt("FID2", shape=(128, 16), dt=I32)
                    nc.vector.tensor_copy(
                        out=_fr(FID2[:], [[2, 4], [1, 2], [8, 2]]),
                        in_=FIDF[:])

                    # bilinear corner weights
                    CXt = newt("CXt")
                    ts(CXt[:], WX[:], -1.0, 1.0, ALU.mult, ALU.add)
                    CYt = newt("CYt")
                    ts(CYt[:], WY[:], -1.0, 1.0, ALU.mult, ALU.add)
                    WA = newt("WA")
                    tt_(WA[:], CXt[:], CYt[:], ALU.mult)
                    WB = newt("WB")
                    tt_(WB[:], WX[:], CYt[:], ALU.mult)
                    WC = newt("WC")
                    tt_(WC[:], CXt[:], WY[:], ALU.mult)
                    WD = newt("WD")
                    tt_(WD[:], WX[:], WY[:], ALU.mult)

                    if dbg and g == 0:
                        nc.sync.dma_start(out=d_idx.ap(), in_=IDX2[:])
                        nc.sync.dma_start(out=d_wa.ap(), in_=WA[:])
                    for jj in range(4):
                        for hh in range(2):
                            boff = 2 * jj * 32 + hh * 16

                            def bsl(tl, bc32=False):
                                dims = ([[32, 2], [1, 16]]
                                        + ([[0, 32]] if bc32 else []))
                                return _fr(tl[:], dims, extra_off=boff)

                            G2 = gatpool.tile([128, 2, N, 128], F32, tag="G2")
                            for k in range(32):
                                nc.gpsimd.indirect_dma_start(
                                    out=_fr(G2[:], [[1, 128]], extra_off=k * 128),
                                    out_offset=None, in_=fp2.ap(),
                                    in_offset=bass.IndirectOffsetOnAxis(
                                        ap=_fr(IDX2[:], [[1, 1]],
                                               extra_off=hh * 128 + jj * 32 + k),
                                        axis=0))
                            if dbg and g == 0 and jj == 0 and hh == 0:
                                nc.sync.dma_start(out=d_g2.ap(),
                                                  in_=_fr(G2[:], [[1, 4096]]))
                            f2 = gatpool.tile([128, 2, 4 * C], F32, tag="f2")
                            for k in range(2):
                                nc.gpsimd.indirect_dma_start(
                                    out=_fr(f2[:], [[1, 128]], extra_off=k * 128),
                                    out_offset=None, in_=fp2.ap(),
                                    in_offset=bass.IndirectOffsetOnAxis(
                                        ap=_fr(FID2[:], [[1, 1]],
                                               extra_off=hh * 8 + jj * 2 + k),
                                        axis=0))

                            f3 = cmppool.tile([128, 2, 3, C], F32, tag="f3")
                            nc.vector.tensor_copy(
                                out=_fr(f3[:], [[96, 2], [1, 32]]),
                                in_=_fr(f2[:], [[128, 2], [1, 32]]))
                            nc.vector.tensor_copy(
                                out=_fr(f3[:], [[96, 2], [1, 24]], extra_off=32),
                                in_=_fr(f2[:], [[128, 2], [1, 24]], extra_off=8))
                            nc.vector.tensor_copy(
                                out=_fr(f3[:], [[96, 2], [1, 8]], extra_off=56),
                                in_=_fr(f2[:], [[128, 2], [1, 8]]))
                            nc.vector.tensor_copy(
                                out=_fr(f3[:], [[96, 2], [1, 16]], extra_off=64),
                                in_=_fr(f2[:], [[128, 2], [1, 16]], extra_off=16))
                            nc.vector.tensor_copy(
                                out=_fr(f3[:], [[96, 2], [1, 16]], extra_off=80),
                                in_=_fr(f2[:], [[128, 2], [1, 16]]))

                            def corner(off):
                                return _fr(G2[:],
                                           [[2048, 2], [128, 16], [1, 32]],
                                           extra_off=off)

                            M1 = cmppool.tile([128, 2, N, C], F32, tag="M1")
                            M2 = cmppool.tile([128, 2, N, C], F32, tag="M2")
                            WARP = cmppool.tile([128, 2, N, C], F32, tag="WARP")
                            tt_(M1[:], corner(0), bsl(WA, True), ALU.mult)
                            tt_(M2[:], corner(64), bsl(WB, True), ALU.mult)
                            tt_(WARP[:], M1[:], M2[:], ALU.add)
                            tt_(M1[:], corner(32), bsl(WC, True), ALU.mult)
                            tt_(WARP[:], WARP[:], M1[:], ALU.add)
                            tt_(M2[:], corner(96), bsl(WD, True), ALU.mult)
                            tt_(WARP[:], WARP[:], M2[:], ALU.add)

                            D3 = cmppool.tile([128, 3072], F32, tag="D3")
                            tt_(_fr(D3[:], [[1536, 2], [512, 3], [32, 16], [1, 32]]),
                                _fr(f3[:], [[96, 2], [32, 3], [0, 16], [1, 32]]),
                                _fr(WARP[:], [[512, 2], [0, 3], [32, 16], [1, 32]]),
                                ALU.subtract)

                            S = smpool.tile([128, 384], F32, tag="S")
                            nc.vector.tensor_reduce(
                                out=S[:], in_=_fr(D3[:], [[8, 384], [1, 8]]),
                                axis=AXL.X, op=ALU.add, apply_absolute_value=True)
                            SMIN = smpool.tile([128, 2, N], F32, tag="SMIN")
                            nc.vector.tensor_reduce(
                                out=SMIN[:],
                                in_=_fr(S[:], [[192, 2], [4, 16], [64, 3], [1, 4]]),
                                axis=AXL.XY, op=ALU.min)
                            if dbg and g == 0 and jj == 0 and hh == 0:
                                nc.sync.dma_start(out=d_smin.ap(),
                                                  in_=_fr(SMIN[:], [[1, 32]]))
                            MM = smpool.tile([128, 2], F32, tag="MM")
                            nc.vector.tensor_reduce(out=MM[:], in_=SMIN[:],
                                                    axis=AXL.X, op=ALU.min)
                            TD = smpool.tile([128, 2, N], F32, tag="TD")
                            tt_(TD[:], SMIN[:], _fr(MM[:], [[1, 2], [0, 16]]),
                                ALU.subtract)
                            E = smpool.tile([128, 2, N], F32, tag="E")
                            nc.scalar.activation(out=E[:], in_=TD[:],
                                                 func=ACTF.Exp, scale=-125.0)
                            SSUM = smpool.tile([128, 2], F32, tag="SSUM")
                            nc.vector.tensor_reduce(out=SSUM[:], in_=E[:],
                                                    axis=AXL.X, op=ALU.add)
                            REC = smpool.tile([128, 2], F32, tag="REC")
                            nc.vector.reciprocal(out=REC[:], in_=SSUM[:])

                            for ax, OT in (('x', OX), ('y', OY)):
                                MXT = smpool.tile([128, 2, N], F32, tag=f"MX{ax}",
                                                  name=f"MX{ax}")
                                tt_(MXT[:],
                                    _fr(OT[:], [[32, 2], [1, 16]],
                                        extra_off=boff),
                                    E[:], ALU.mult)
                                SX = smpool.tile([128, 2], F32, tag=f"SX{ax}",
                                                 name=f"SX{ax}")
                                nc.vector.tensor_reduce(out=SX[:], in_=MXT[:],
                                                        axis=AXL.X, op=ALU.add)
                                VX = smpool.tile([128, 2], F32, tag=f"VX{ax}",
                                                 name=f"VX{ax}")
                                tt_(VX[:], SX[:], REC[:], ALU.mult)
                                dst = _fr(OUTT[(ax, hh)][:], [[1, 2]],
                                          extra_off=g * 8 + 2 * jj)
                                if ax == 'x':
                                    xc = xc0 if hh == 0 else xc1
                                    P1 = smpool.tile([128, 2], F32, tag="P1",
                                                     name="P1")
                                    ts(P1[:], VX[:], xc[:], float(W - 1),
                                       ALU.add, ALU.min)
                                    ts(dst, P1[:], 0.0, xc[:], ALU.max,
                                       ALU.subtract)
                                else:
                                    yfs = _fr(YF[:], [[32, 2]], extra_off=boff)
                                    P1 = smpool.tile([128, 2], F32, tag="P1y",
                                                     name="P1y")
                                    tt_(P1[:], VX[:], yfs, ALU.add)
                                    ts(P1[:], P1[:], 0.0, float(H - 1),
                                       ALU.max, ALU.min)
                                    tt_(dst, P1[:], yfs, ALU.subtract)

            # ---------------- outputs ----------------
            for ax, ot in (('x', outx), ('y', outy)):
                for hh in range(2):
                    po = psA.tile([128, 128], F32, tag="po", name="po")
                    nc.tensor.transpose(out=po[:], in_=OUTT[(ax, hh)][:],
                                        identity=idn[:])
                    so = ixspool.tile([128, 128], F32, tag="so", name="so")
                    nc.vector.tensor_copy(out=so[:], in_=po[:])
                    nc.sync.dma_start(out=ot.ap()[:, hh * 128:(hh + 1) * 128],
                                      in_=so[:])

    nc.compile()
    return nc


_NC_CACHE = None


def _get_module():
    global _NC_CACHE
    if _NC_CACHE is None:
        _NC_CACHE = build_module()
    return _NC_CACHE


def make_in_maps(features, offset_x, offset_y):
    features = np.ascontiguousarray(features, dtype=np.float32)
    offset_x = np.ascontiguousarray(offset_x, dtype=np.float32)
    offset_y = np.ascontiguousarray(offset_y, dtype=np.float32)
    in_maps = []
    for core in range(NCORES):
        b = core // 2
        r0 = (core % 2) * 128
        in_maps.append({
            "feat": features[b].reshape(C, HW).copy(),
            "offx": offset_x[b, :, r0:r0 + 128, :].copy(),
            "offy": offset_y[b, :, r0:r0 + 128, :].copy(),
            "rbase": np.full((128, 1), float(r0), np.float32),
        })
    return in_maps


def assemble(results):
    B = NCORES // 2
    fx = np.zeros((B, 1, H, W), np.float32)
    fy = np.zeros((B, 1, H, W), np.float32)
    for core in range(NCORES):
        b = core // 2
        r0 = (core % 2) * 128
        fx[b, 0, r0:r0 + 128, :] = results[core]["outx"]
        fy[b, 0, r0:r0 + 128, :] = results[core]["outy"]
    return fx, fy


def kernel(features, offset_x, offset_y, left_x, left_y, roll0, roll1,
           group_size, _trace=False):
    assert int(roll0) == 8 and int(roll1) == 16 and int(group_size) == 8
    from concourse import bass_utils
    nc = _get_module()
    in_maps = make_in_maps(features, offset_x, offset_y)
    res = bass_utils.run_bass_kernel_spmd(nc, in_maps,
                                          core_ids=list(range(NCORES)),
                                          trace=_trace)
    fx, fy = assemble(res.results)
    if _trace:
        return (fx, fy), res
    return fx, fy
